# revision 19
# baseline (speedup 1.0000x reference)
"""Trainium2 Bass kernel for batched cross-attention:

    score[b,e,t] = sum_d enc[b,e,d] * dec[b,t,d]
    attn = softmax(score, axis=e)
    context[b,t,d] = sum_e enc[b,e,d] * attn[b,e,t]
    out = concat([dec, context], axis=-1)          # [B, T, 2D]

Sharding: batch (B=8) across 8 NeuronCores, one batch element per core.

Per-core algorithm (statically unrolled, T=2048, D=512):
  - score matmuls in bf16 (eT, dT transposed copies); context matmuls
    bf16 with the softmax denominator fused as a ones-column of the E
    copy (two PSUM banks, N=256|257 per k), so no separate ones-matmuls
    or sum passes exist.
  - all 128x128 transposes are bf16 identity-matmuls on the PE (the
    DMA xbar path acquires the DMA engines exclusively at ~1.3us per
    chunk, measured, so it serializes the input streams and loses).
  - HAM warmup: 30 throwaway 128-col matmuls at kernel start plus one
    keep-warm matmul per E chunk in tb=0, so the PE clock gate reaches
    8/8 (2.4GHz) early and stays there through the transpose-heavy
    prologue (identity transposes do not count as PE-busy for HAM).
  - queue plan (each DMA issue costs ~600-700ns of engine time): E
    chunks 0-1 on sync (fast HWDGE landing, they gate the first
    score), E2-15 on gpsimd; D0-3 on sync up front, D4-15 on gpsimd
    per score block; D casts + E casts + psum drains + normalize on
    DVE; exps on scalar (exp-only so it keeps pace with 2-deep score
    PSUM); dec-half stores on gpsimd mid-kernel; ctx stores alternate
    sync/scalar; the final tile's normalize+store is quartered and
    pipelined across both HWDGE queues.
  - fixed softmax shift exp(s - 100) (mathematically exact softmax;
    scores ~ N(0, 512) so no overflow).
"""

import numpy as np

_B, _T, _D = 8, 2048, 512
_NCORES = 8

_cached_nc = None


def _build():
    global _cached_nc
    if _cached_nc is not None:
        return _cached_nc

    import concourse.tile as tile
    from concourse import bacc, mybir
    from concourse.masks import make_identity

    f32 = mybir.dt.float32
    bf16 = mybir.dt.bfloat16
    T, D = _T, _D
    EC = T // 128   # 16 encoder chunks of 128
    DC = D // 128   # 4 d chunks of 128
    TB = 512        # decoder-time block for scores
    NTB = T // TB   # 4
    NTS = T // 128  # 16 context t-subs
    SHIFT = -100.0
    NWARM = 30
    Exp = mybir.ActivationFunctionType.Exp

    nc = bacc.Bacc("TRN2", target_bir_lowering=False, debug=False,
                   num_devices=_NCORES)
    enc = nc.dram_tensor("encoder_outputs", [T, D], f32, kind="ExternalInput")
    dec = nc.dram_tensor("decoder_outputs", [T, D], f32, kind="ExternalInput")
    out = nc.dram_tensor("out", [T, 2 * D], f32, kind="ExternalOutput")

    with tile.TileContext(nc) as tc:
        with (
            tc.tile_pool(name="persist", bufs=1) as persist,
            tc.tile_pool(name="e_stage", bufs=6) as e_stage,
            tc.tile_pool(name="d_stage", bufs=EC) as d_stage,
            tc.tile_pool(name="d_cast", bufs=3) as d_cast,
            tc.tile_pool(name="copool", bufs=3) as copool,
            tc.tile_pool(name="small", bufs=4) as small,
            tc.tile_pool(name="ps_t", bufs=2, space="PSUM") as ps_t,
            tc.tile_pool(name="ps_s", bufs=2, space="PSUM") as ps_s,
            tc.tile_pool(name="ps_c", bufs=2, space="PSUM") as ps_c,
        ):
            eT = persist.tile([128, DC, T], bf16)      # E^T [d, e]
            dT = persist.tile([128, DC, T], bf16)      # D^T [d, t]
            e_ctx = persist.tile([128, EC, D + 1], bf16)  # E natural + ones
            A = persist.tile([128, EC, T], bf16)       # attn weights [e, t]
            nbias = persist.tile([128, 1], f32)
            ident = persist.tile([128, 128], f32)
            identb = persist.tile([128, 128], bf16)

            est = {}
            dst = {}

            def load_e(k):
                st = e_stage.tile([128, D], f32, tag="est")
                q = nc.sync if k < 8 else nc.gpsimd
                q.dma_start(st[:], enc[k * 128:(k + 1) * 128, :])
                est[k] = st

            def load_d(j, q):
                st = d_stage.tile([128, D], f32, tag="dst")
                q.dma_start(st[:], dec[j * 128:(j + 1) * 128, :])
                dst[j] = st

            # identity + memsets FIRST: make_identity runs on gpsimd and
            # must not queue behind the E-load issues; everything
            # downstream (transposes, warmups) depends on identb.
            nc.vector.memset(nbias[:], SHIFT)
            nc.vector.memset(e_ctx[:, :, D:D + 1], 1.0)
            make_identity(nc, ident[:])
            nc.vector.tensor_copy(identb[:], ident[:])

            # loads next so the DMA queues start streaming during the
            # warmup matmuls: D block 0 + E0-1 on sync, E2-15 on gpsimd
            # (parallel queues).
            for j in range(DC):
                load_d(j, nc.sync)
            for k in range(EC):
                load_e(k)

            # ---- HAM warmup: keep the PE busy while DMAs stream so the
            # clock gate is at 8/8 when the first real matmul issues.
            # (identity transposes don't count as PE-busy for HAM, so
            # these are regular matmuls.)
            for _ in range(NWARM):
                wp = ps_s.tile([128, TB], f32, tag="S")
                nc.tensor.matmul(wp[:, 0:128], identb[:], identb[:],
                                 start=True, stop=True)

            def keepwarm():
                # transposes do not count as PE-busy for the HAM clock
                # gate; sprinkle a real matmul through transpose-heavy
                # stretches so the PE stays at 2.4GHz.
                wp = ps_s.tile([128, TB], f32, tag="S")
                nc.tensor.matmul(wp[:, 0:128], identb[:], identb[:],
                                 start=True, stop=True)

            def proc_e(k):
                """cast bf16 into e_ctx, PE-transpose into eT."""
                nc.vector.tensor_copy(e_ctx[:, k, 0:D], est[k][:])
                pst = ps_t.tile([128, DC, 128], bf16, tag="tP")
                for j in range(DC):
                    nc.tensor.transpose(pst[:, j, :],
                                        e_ctx[:, k, j * 128:(j + 1) * 128],
                                        identb[:])
                nc.vector.tensor_copy(eT[:, :, k * 128:(k + 1) * 128],
                                      pst[:])

            def proc_d(j):
                """cast bf16 (DVE), PE-transpose into dT."""
                dc = d_cast.tile([128, D], bf16, tag="dc")
                nc.vector.tensor_copy(dc[:], dst[j][:])
                pst = ps_t.tile([128, DC, 128], bf16, tag="tP")
                for i in range(DC):
                    nc.tensor.transpose(pst[:, i, :],
                                        dc[:, i * 128:(i + 1) * 128],
                                        identb[:])
                nc.vector.tensor_copy(dT[:, :, j * 128:(j + 1) * 128],
                                      pst[:])

            def score(tb, k):
                s_ps = ps_s.tile([128, TB], f32, tag="S")
                for dj in range(DC):
                    nc.tensor.matmul(
                        s_ps[:],
                        eT[:, dj, k * 128:(k + 1) * 128],
                        dT[:, dj, tb * TB:(tb + 1) * TB],
                        start=(dj == 0), stop=(dj == DC - 1),
                    )
                nc.scalar.activation(A[:, k, tb * TB:(tb + 1) * TB], s_ps[:],
                                     Exp, bias=nbias[:])

            def ctx(ts):
                # two PSUM banks: bank0 = d cols 0:256, bank1 = d cols
                # 256:512 plus the fused softmax-denominator at col 256
                # (matmul outputs cannot cross a bank boundary).
                c_ps = ps_c.tile([128, 2, 512], f32, tag="C")
                for k in range(EC):
                    lhsT = A[:, k, ts * 128:(ts + 1) * 128]
                    nc.tensor.matmul(
                        c_ps[:, 0, 0:256], lhsT, e_ctx[:, k, 0:256],
                        start=(k == 0), stop=(k == EC - 1),
                    )
                    nc.tensor.matmul(
                        c_ps[:, 1, 0:257], lhsT, e_ctx[:, k, 256:D + 1],
                        start=(k == 0), stop=(k == EC - 1),
                    )
                recip = small.tile([128, 1], f32, tag="r")
                nc.vector.reciprocal(recip[:], c_ps[:, 1, 256:257])
                c_sb = copool.tile([128, D], f32, tag="co")
                row0 = ts * 128
                if ts == NTS - 1:
                    # pipeline the last tile: quarter the normalize and
                    # interleave each quarter's store on alternating
                    # HWDGE queues so the epilogue tail is short
                    for qi in range(4):
                        lo = qi * 128
                        bank, bo = (0, lo) if qi < 2 else (1, lo - 256)
                        nc.vector.tensor_scalar_mul(
                            c_sb[:, lo:lo + 128],
                            c_ps[:, bank, bo:bo + 128], recip[:])
                        q = nc.sync if qi % 2 == 0 else nc.scalar
                        q.dma_start(out[row0:row0 + 128, D + lo:D + lo + 128],
                                    c_sb[:, lo:lo + 128])
                else:
                    nc.vector.tensor_scalar_mul(c_sb[:, 0:256],
                                                c_ps[:, 0, 0:256], recip[:])
                    nc.vector.tensor_scalar_mul(c_sb[:, 256:D],
                                                c_ps[:, 1, 0:256], recip[:])
                    q = nc.sync if ts % 2 == 0 else nc.scalar
                    q.dma_start(out[row0:row0 + 128, D:2 * D], c_sb[:])

            # ---- prologue processing (first tiles already loading) ----
            for j in range(DC):
                proc_d(j)
            proc_e(0)

            # ---- score phase: (t-block, e-chunk) with dj inner ----
            for tb in range(NTB):
                if tb > 0:
                    for j in range(DC * tb, DC * (tb + 1)):
                        load_d(j, nc.gpsimd)
                        proc_d(j)
                    if tb >= 2:
                        # dec half of the output: 8 stores per block so
                        # they run on gpsimd mid-kernel, clear of both
                        # the D-load deadlines and the final drain
                        for j in range(8 * (tb - 2), 8 * (tb - 1)):
                            nc.gpsimd.dma_start(
                                out[j * 128:(j + 1) * 128, 0:D], dst[j][:])
                for k in range(EC):
                    if tb == 0 and k > 0:
                        proc_e(k)
                        keepwarm()
                    score(tb, k)

            # ---- context phase ----
            for ts in range(NTS):
                ctx(ts)

    nc.compile()
    _cached_nc = nc
    return nc


def kernel(encoder_outputs, decoder_outputs):
    from concourse.bass_utils import run_bass_kernel_spmd

    nc = _build()
    enc = np.ascontiguousarray(encoder_outputs, dtype=np.float32)
    dec = np.ascontiguousarray(decoder_outputs, dtype=np.float32)
    in_maps = [
        {"encoder_outputs": enc[i], "decoder_outputs": dec[i]}
        for i in range(_NCORES)
    ]
    res = run_bass_kernel_spmd(nc, in_maps, core_ids=list(range(_NCORES)))
    return np.stack([r["out"] for r in res.results], axis=0)
